# revision 26
# baseline (speedup 1.0000x reference)
"""Trainium2 Bass kernel for CantorGlobalAttention.

Math (per direction d, expert e, batch b, patch p):
  s_w   = Ksum[d, routes[e,w], b] * scale * beta[e,w]      (w = 0..2)
  v_w   = Vmean[d, routes[e,w], b]
  out_d = softmax_w(q * s_w) . v
Final output = mean over d, laid out [B, E*P].

Since the softmax is over only W=3 scalars that multiply the same q, it
collapses to an elementwise function of q with 6 per-row constants:
  sg  = sigmoid(q*(s0-s1)) = 0.5 + 0.5*tanh(q*(s0-s1)/2)
  A   = v1 + (v0-v1)*sg                       (exact 2-way softmax of {0,1})
  s2  = 1/(1 + e^{q*(s0-s2)} + e^{q*(s1-s2)}) (true 3-way weight of w=2)
  out_d = A + (v2 - A)*s2
Rewritten with tanh (so all ACT funcs live in one table set):
  H  = 0.5*(v0-v1);  Cp = (v2-v1) - H;  T = Cp - H*tanh(q*d01h)
  out_d = v2 - T + T*s2
Accumulated over d on the PE via identity matmuls into PSUM:
  OUT = 0.2 * (sum_d (z_d - T_d)) + 0.2*sum_d v2_d,   z = T*s2

All the per-row constants are produced on-device by tiny one-hot matmuls
(host-built gather matrices folded with scale/beta/0.5 factors) applied to
Ksum / Vsum columns (one col per direction).

Sharding: data-parallel over batch (dim 2), 8 cores x 8 batches.
"""

import numpy as np
from contextlib import ExitStack

import concourse.bass as bass
import concourse.bacc as bacc
import concourse.tile as tile
from concourse import mybir
from concourse import bass_utils

F32 = mybir.dt.float32
AF = mybir.ActivationFunctionType
OP = mybir.AluOpType

D, E, B, P = 5, 16, 64, 4096
W = 3
NCORES = 8
BC = B // NCORES          # 8 batches per core
R = E * BC                # 128 rows = partitions, r = e*BC + b
EXPERT_DIM = 128

CKV = 1024                # K/V reduce chunk (cols)
CQ = 2048                 # Q-phase chunk (cols)
MMF = 512                 # matmul max free dim

CLAMP = 1e37              # keep reciprocal_approx_fast input finite


def _build_bass():
    nc = bacc.Bacc("TRN2", debug=False, num_devices=NCORES)
    # chunk-major layouts: every tile transfer is one contiguous DRAM block
    q = nc.dram_tensor("q", [D, P // CQ, R, CQ], F32,
                       kind="ExternalInput").ap()
    k = nc.dram_tensor("k", [D, P // CKV, R, CKV], F32,
                       kind="ExternalInput").ap()
    v = nc.dram_tensor("v", [D, P // CKV, R, CKV], F32,
                       kind="ExternalInput").ap()
    # 6 gather matrices (pre-transposed for lhsT) + I and -I
    mats = nc.dram_tensor("mats", [8, R, R], F32, kind="ExternalInput").ap()
    out = nc.dram_tensor("out", [P // CQ, R, CQ], F32,
                         kind="ExternalOutput").ap()

    with ExitStack() as ctx:
        tc = ctx.enter_context(tile.TileContext(nc))
        _body(ctx, tc, q, k, v, mats, out)
    if not nc.is_finalized():
        nc.finalize()
    return nc


def _body(ctx, tc, q, k, v, mats, out):
    nc = tc.nc
    singles = ctx.enter_context(tc.tile_pool(name="singles", bufs=1))

    # Long-lived Q-phase pools are created BEFORE the short-lived K/V pool:
    # the stack allocator then never hands the K/V zone to Q-phase tiles,
    # which would serialize the Q pipeline behind the last V reduce
    # (released-zone dependency).
    qpool = ctx.enter_context(tc.tile_pool(name="qp", bufs=5))
    work = ctx.enter_context(tc.tile_pool(name="work", bufs=3))
    th_pool = ctx.enter_context(tc.tile_pool(name="thp", bufs=3))
    outp = ctx.enter_context(tc.tile_pool(name="outp", bufs=2))

    # Alternate big loads across the two HWDGE queue sets (SP + Activation)
    # so whatever is first in program order gets the full HBM bandwidth.
    dma_i = [0]

    def load(dst, src_ap):
        eng = nc.sync if dma_i[0] % 2 == 0 else nc.scalar
        dma_i[0] += 1
        return eng.dma_start(out=dst, in_=src_ap)

    # --- constants to SBUF ---
    mat_sb = []
    for i in range(8):
        m = singles.tile([R, R], F32, tag=f"mat{i}")
        load(m, mats[i, :, :])
        mat_sb.append(m)
    (m_d01h, m_d02, m_d12, m_hn, m_cp, m_v2, m_pos, m_neg) = mat_sb

    ksum = singles.tile([R, D], F32, tag="ksum")
    vsum = singles.tile([R, D], F32, tag="vsum")
    nkv = P // CKV

    with tc.tile_pool(name="kv", bufs=4) as kv_pool, \
         tc.tile_pool(name="part", bufs=4) as part_pool:

        def reduce_phase(name, src, dst):
            loads = []
            for d in range(D):
                parts = part_pool.tile([R, nkv], F32, tag=f"{name}p")
                for c in range(nkv):
                    t = kv_pool.tile([R, CKV], F32, tag=name)
                    loads.append(load(t, src[d, c, :, :]))
                    nc.vector.tensor_reduce(out=parts[:, c:c + 1], in_=t,
                                            axis=mybir.AxisListType.X,
                                            op=OP.add)
                nc.vector.tensor_add(parts[:, 0:1], parts[:, 0:1],
                                     parts[:, 1:2])
                nc.vector.tensor_add(parts[:, 2:3], parts[:, 2:3],
                                     parts[:, 3:4])
                nc.vector.tensor_add(dst[:, d:d + 1], parts[:, 0:1],
                                     parts[:, 2:3])
            return loads

        # K first (ksum gates the whole Q pipeline)
        k_loads = reduce_phase("k", k, ksum)

        # prefetch the first Q chunk of every direction; gate behind the K
        # tail so K gets the full HBM bandwidth (HW DGE interleaves live
        # transfers); queue-FIFO puts V behind these on both streams
        qpre = []
        for d in range(D):
            qt = qpool.tile([R, CQ], F32, tag="q")
            ld = load(qt, q[d, 0, :, :])
            if d < 2:
                bass._add_dep_helper(ld.ins, k_loads[-1 - d].ins, sync=True,
                                     reason="bandwidth: K before Q prefetch")
            qpre.append(qt)

        reduce_phase("v", v, vsum)

    # --- prelude: per-row constants via one-hot matmuls ---
    d01h = singles.tile([R, D], F32, tag="d01h")
    d02 = singles.tile([R, D], F32, tag="d02")
    d12 = singles.tile([R, D], F32, tag="d12")
    hn = singles.tile([R, D], F32, tag="hn")
    cp = singles.tile([R, D], F32, tag="cp")
    v2 = singles.tile([R, D], F32, tag="v2")
    c2s = singles.tile([R, 1], F32, tag="c2s")

    with tc.tile_pool(name="prep", bufs=2, space="PSUM") as pre_psum:
        for lhsT, rhs, dst in ((m_d01h, ksum, d01h), (m_d02, ksum, d02),
                               (m_d12, ksum, d12), (m_hn, vsum, hn),
                               (m_cp, vsum, cp), (m_v2, vsum, v2)):
            pt = pre_psum.tile([R, D], F32, tag="pre")
            nc.tensor.matmul(pt, lhsT, rhs, start=True, stop=True)
            nc.vector.tensor_copy(dst, pt)
    c2r = singles.tile([R, 1], F32, tag="c2r")
    nc.vector.tensor_reduce(out=c2r, in_=v2, axis=mybir.AxisListType.X,
                            op=OP.add)
    nc.vector.tensor_scalar_mul(c2s, c2r, 1.0 / D)

    # --- Q phase ---
    acc_pool = ctx.enter_context(tc.tile_pool(name="accp", bufs=2,
                                              space="PSUM"))
    nq = P // CQ
    for c in range(nq):
        acc = acc_pool.tile([R, CQ], F32, tag="acc")
        for d in range(D):
            if c == 0:
                qt = qpre[d]
            else:
                qt = qpool.tile([R, CQ], F32, tag="q")
                # SP stream only: a slot-waiting trigger here must not block
                # ACT compute
                nc.sync.dma_start(out=qt, in_=q[d, c, :, :])
            th = th_pool.tile([R, CQ], F32, tag="th")
            nc.scalar.activation(out=th, in_=qt, func=AF.Tanh,
                                 scale=d01h[:, d:d + 1])
            ea = work.tile([R, CQ], F32, tag="ea")
            nc.scalar.activation(out=ea, in_=qt, func=AF.Exp,
                                 scale=d02[:, d:d + 1])
            eb = work.tile([R, CQ], F32, tag="eb")
            nc.scalar.activation(out=eb, in_=qt, func=AF.Exp,
                                 scale=d12[:, d:d + 1])
            # s2 = 1/min(1 + ea + eb, CLAMP); reuse ea/eb storage in place
            nc.gpsimd.tensor_tensor(ea, ea, eb, OP.add)
            nc.vector.tensor_scalar(ea, ea, 1.0, CLAMP, OP.add, OP.min)
            nc.vector.reciprocal_approx_fast(out=eb, in_=ea)  # eb := s2
            # T = Cp - H*th ; alternate engine for load balance
            tt = work.tile([R, CQ], F32, tag="tt")
            if (c * D + d) % 2 == 0:
                nc.scalar.activation(out=tt, in_=th, func=AF.Identity,
                                     scale=hn[:, d:d + 1], bias=cp[:, d:d + 1])
            else:
                nc.vector.tensor_scalar(tt, th, hn[:, d:d + 1], cp[:, d:d + 1],
                                        OP.mult, OP.add)
            # z = s2*T (into th's storage); PE accumulates z - T
            if (c * D + d) % 4 == 0:
                nc.gpsimd.tensor_tensor(th, eb, tt, OP.mult)  # th := z
            else:
                nc.vector.tensor_mul(th, eb, tt)  # th := z
            for pc in range(CQ // MMF):
                sl = slice(pc * MMF, (pc + 1) * MMF)
                nc.tensor.matmul(acc[:, sl], m_pos, th[:, sl],
                                 start=(d == 0), stop=False)
                nc.tensor.matmul(acc[:, sl], m_neg, tt[:, sl],
                                 start=False, stop=(d == D - 1))
        osb = outp.tile([R, CQ], F32, tag="osb")
        nc.scalar.activation(out=osb, in_=acc, func=AF.Identity,
                             scale=1.0 / D, bias=c2s[:, 0:1])
        nc.scalar.dma_start(out=out[c, :, :], in_=osb)


def _host_constants(betas, temperature, routes):
    """Build the 6 gather matrices (+-I) from the tiny replicated inputs."""
    betas = np.asarray(betas, dtype=np.float32)
    routes = np.asarray(routes).astype(np.int64)
    temp = np.abs(np.asarray(temperature, dtype=np.float32).reshape(-1)[0])
    scale = np.float32(1.0) / (np.sqrt(np.float32(EXPERT_DIM)) * temp)

    self_idx = np.arange(E)
    gate = np.where(
        routes == self_idx[:, None], np.float32(1.0),
        (np.float32(1.0) / (np.float32(1.0) +
                            np.exp(-betas[self_idx[:, None], routes]))),
    ).astype(np.float32)  # [E, W]

    A = np.zeros((W, R, R), dtype=np.float32)   # s_w gather (scale*beta folded)
    G = np.zeros((W, R, R), dtype=np.float32)   # v_w gather (1/P folded)
    rows = np.arange(R)
    e_of_r = rows // BC
    b_of_r = rows % BC
    for w in range(W):
        cols = routes[e_of_r, w] * BC + b_of_r
        A[w, rows, cols] += scale * gate[e_of_r, w]
        G[w, rows, cols] += np.float32(1.0 / P)

    m_d01h = 0.5 * (A[0] - A[1])
    m_d02 = A[0] - A[2]
    m_d12 = A[1] - A[2]
    m_h = 0.5 * (G[0] - G[1])
    m_hn = -m_h
    m_cp = (G[2] - G[1]) - m_h
    m_v2 = G[2]
    eye = np.eye(R, dtype=np.float32)
    mats = np.stack([m_d01h.T, m_d02.T, m_d12.T, m_hn.T, m_cp.T, m_v2.T,
                     eye, -eye]).astype(np.float32)
    return np.ascontiguousarray(mats)


_CACHE = {}


def kernel(Q, K, V, betas, temperature, routes, num_patches):
    Q = np.asarray(Q, dtype=np.float32)
    K = np.asarray(K, dtype=np.float32)
    V = np.asarray(V, dtype=np.float32)
    mats = _host_constants(betas, temperature, routes)

    if "nc" not in _CACHE:
        _CACHE["nc"] = _build_bass()
    nc = _CACHE["nc"]

    def shard(X, C):
        # [D,E,B,P] batch-slice -> chunk-major [D, P//C, R, C], contiguous
        outs = []
        for i in range(NCORES):
            sl = X[:, :, i * BC:(i + 1) * BC, :].reshape(D, R, P // C, C)
            outs.append(np.ascontiguousarray(sl.transpose(0, 2, 1, 3)))
        return outs

    qs, ks, vs = shard(Q, CQ), shard(K, CKV), shard(V, CKV)
    in_maps = [{"q": qs[i], "k": ks[i], "v": vs[i], "mats": mats}
               for i in range(NCORES)]

    res = bass_utils.run_bass_kernel_spmd(nc, in_maps,
                                          core_ids=list(range(NCORES)))
    _CACHE["last"] = res
    # device out: [P//CQ, R, CQ] with r = e*BC + b -> [BC, E*P]
    full = np.empty((B, E * P), dtype=np.float32)
    nq = P // CQ
    for i in range(NCORES):
        o = res.results[i]["out"].reshape(nq, E, BC, CQ)
        full[i * BC:(i + 1) * BC] = (
            o.transpose(2, 1, 0, 3).reshape(BC, E * P))
    return full


# revision 27
# speedup vs baseline: 1.0589x; 1.0589x over previous
"""Trainium2 Bass kernel for CantorGlobalAttention.

Math (per direction d, expert e, batch b, patch p):
  s_w   = Ksum[d, routes[e,w], b] * scale * beta[e,w]      (w = 0..2)
  v_w   = Vmean[d, routes[e,w], b]
  out_d = softmax_w(q * s_w) . v
Final output = mean over d, laid out [B, E*P].

Since the softmax is over only W=3 scalars that multiply the same q, it
collapses to an elementwise function of q with 6 per-row constants:
  sg  = sigmoid(q*(s0-s1)) = 0.5 + 0.5*tanh(q*(s0-s1)/2)
  A   = v1 + (v0-v1)*sg                       (exact 2-way softmax of {0,1})
  s2  = 1/(1 + e^{q*(s0-s2)} + e^{q*(s1-s2)}) (true 3-way weight of w=2)
  out_d = A + (v2 - A)*s2
Rewritten with tanh (so all ACT funcs live in one table set):
  H  = 0.5*(v0-v1);  Cp = (v2-v1) - H;  T = Cp - H*tanh(q*d01h)
  out_d = v2 - T + T*s2
Accumulated over d on the PE via identity matmuls into PSUM:
  OUT = 0.2 * (sum_d (z_d - T_d)) + 0.2*sum_d v2_d,   z = T*s2

All the per-row constants are produced on-device by tiny one-hot matmuls
(host-built gather matrices folded with scale/beta/0.5 factors) applied to
Ksum / Vsum columns (one col per direction).

Sharding: data-parallel over batch (dim 2), 8 cores x 8 batches.
"""

import numpy as np
from contextlib import ExitStack

import concourse.bass as bass
import concourse.bacc as bacc
import concourse.tile as tile
from concourse import mybir
from concourse import bass_utils

F32 = mybir.dt.float32
AF = mybir.ActivationFunctionType
OP = mybir.AluOpType

D, E, B, P = 5, 16, 64, 4096
W = 3
NCORES = 8
BC = B // NCORES          # 8 batches per core
R = E * BC                # 128 rows = partitions, r = e*BC + b
EXPERT_DIM = 128

CKV = 1024                # K/V reduce chunk (cols)
CQ = 2048                 # Q-phase chunk (cols)
MMF = 512                 # matmul max free dim

CLAMP = 1e37              # keep reciprocal_approx_fast input finite


def _build_bass():
    nc = bacc.Bacc("TRN2", debug=False, num_devices=NCORES)
    # chunk-major layouts: every tile transfer is one contiguous DRAM block
    q = nc.dram_tensor("q", [D, P // CQ, R, CQ], F32,
                       kind="ExternalInput").ap()
    k = nc.dram_tensor("k", [D, P // CKV, R, CKV], F32,
                       kind="ExternalInput").ap()
    v = nc.dram_tensor("v", [D, P // CKV, R, CKV], F32,
                       kind="ExternalInput").ap()
    # 6 gather matrices (pre-transposed for lhsT) + I and -I
    mats = nc.dram_tensor("mats", [8, R, R], F32, kind="ExternalInput").ap()
    out = nc.dram_tensor("out", [P // CQ, R, CQ], F32,
                         kind="ExternalOutput").ap()

    with ExitStack() as ctx:
        tc = ctx.enter_context(tile.TileContext(nc))
        _body(ctx, tc, q, k, v, mats, out)
    if not nc.is_finalized():
        nc.finalize()
    return nc


def _body(ctx, tc, q, k, v, mats, out):
    nc = tc.nc
    singles = ctx.enter_context(tc.tile_pool(name="singles", bufs=1))

    # Long-lived Q-phase pools are created BEFORE the short-lived K/V pool:
    # the stack allocator then never hands the K/V zone to Q-phase tiles,
    # which would serialize the Q pipeline behind the last V reduce
    # (released-zone dependency).
    qpool = ctx.enter_context(tc.tile_pool(name="qp", bufs=5))
    work = ctx.enter_context(tc.tile_pool(name="work", bufs=3))
    th_pool = ctx.enter_context(tc.tile_pool(name="thp", bufs=3))
    outp = ctx.enter_context(tc.tile_pool(name="outp", bufs=2))

    # All input loads go through the SP trigger stream: its FIFO gives
    # strict priority ordering (K -> Q prefetch -> V -> Q rest) and the DGE
    # spreads transfers over all 16 HW queues regardless of issuing engine.
    def load(dst, src_ap):
        return nc.sync.dma_start(out=dst, in_=src_ap)

    # --- constants to SBUF ---
    mat_sb = []
    for i in range(8):
        m = singles.tile([R, R], F32, tag=f"mat{i}")
        load(m, mats[i, :, :])
        mat_sb.append(m)
    (m_d01h, m_d02, m_d12, m_hn, m_cp, m_v2, m_pos, m_neg) = mat_sb

    ksum = singles.tile([R, D], F32, tag="ksum")
    vsum = singles.tile([R, D], F32, tag="vsum")
    nkv = P // CKV

    with tc.tile_pool(name="kv", bufs=4) as kv_pool, \
         tc.tile_pool(name="part", bufs=4) as part_pool:

        def reduce_phase(name, src, dst):
            loads = []
            for d in range(D):
                parts = part_pool.tile([R, nkv], F32, tag=f"{name}p")
                for c in range(nkv):
                    t = kv_pool.tile([R, CKV], F32, tag=name)
                    loads.append(load(t, src[d, c, :, :]))
                    nc.vector.tensor_reduce(out=parts[:, c:c + 1], in_=t,
                                            axis=mybir.AxisListType.X,
                                            op=OP.add)
                nc.vector.tensor_add(parts[:, 0:1], parts[:, 0:1],
                                     parts[:, 1:2])
                nc.vector.tensor_add(parts[:, 2:3], parts[:, 2:3],
                                     parts[:, 3:4])
                nc.vector.tensor_add(dst[:, d:d + 1], parts[:, 0:1],
                                     parts[:, 2:3])
            return loads

        # K first (ksum gates the whole Q pipeline)
        k_loads = reduce_phase("k", k, ksum)

        # prefetch the first Q chunk of every direction (after K in FIFO)
        qpre = []
        for d in range(D):
            qt = qpool.tile([R, CQ], F32, tag="q")
            load(qt, q[d, 0, :, :])
            qpre.append(qt)

        reduce_phase("v", v, vsum)

    # --- prelude: per-row constants via one-hot matmuls ---
    d01h = singles.tile([R, D], F32, tag="d01h")
    d02 = singles.tile([R, D], F32, tag="d02")
    d12 = singles.tile([R, D], F32, tag="d12")
    hn = singles.tile([R, D], F32, tag="hn")
    cp = singles.tile([R, D], F32, tag="cp")
    v2 = singles.tile([R, D], F32, tag="v2")
    c2s = singles.tile([R, 1], F32, tag="c2s")

    with tc.tile_pool(name="prep", bufs=2, space="PSUM") as pre_psum:
        for lhsT, rhs, dst in ((m_d01h, ksum, d01h), (m_d02, ksum, d02),
                               (m_d12, ksum, d12), (m_hn, vsum, hn),
                               (m_cp, vsum, cp), (m_v2, vsum, v2)):
            pt = pre_psum.tile([R, D], F32, tag="pre")
            nc.tensor.matmul(pt, lhsT, rhs, start=True, stop=True)
            nc.vector.tensor_copy(dst, pt)
    c2r = singles.tile([R, 1], F32, tag="c2r")
    nc.vector.tensor_reduce(out=c2r, in_=v2, axis=mybir.AxisListType.X,
                            op=OP.add)
    nc.vector.tensor_scalar_mul(c2s, c2r, 1.0 / D)

    # --- Q phase ---
    acc_pool = ctx.enter_context(tc.tile_pool(name="accp", bufs=2,
                                              space="PSUM"))
    nq = P // CQ
    for c in range(nq):
        acc = acc_pool.tile([R, CQ], F32, tag="acc")
        for d in range(D):
            if c == 0:
                qt = qpre[d]
            else:
                qt = qpool.tile([R, CQ], F32, tag="q")
                # SP stream only: a slot-waiting trigger here must not block
                # ACT compute
                nc.sync.dma_start(out=qt, in_=q[d, c, :, :])
            th = th_pool.tile([R, CQ], F32, tag="th")
            nc.scalar.activation(out=th, in_=qt, func=AF.Tanh,
                                 scale=d01h[:, d:d + 1])
            ea = work.tile([R, CQ], F32, tag="ea")
            nc.scalar.activation(out=ea, in_=qt, func=AF.Exp,
                                 scale=d02[:, d:d + 1])
            eb = work.tile([R, CQ], F32, tag="eb")
            nc.scalar.activation(out=eb, in_=qt, func=AF.Exp,
                                 scale=d12[:, d:d + 1])
            # s2 = 1/min(1 + ea + eb, CLAMP); reuse ea/eb storage in place
            nc.gpsimd.tensor_tensor(ea, ea, eb, OP.add)
            nc.vector.tensor_scalar(ea, ea, 1.0, CLAMP, OP.add, OP.min)
            nc.vector.reciprocal_approx_fast(out=eb, in_=ea)  # eb := s2
            # T = Cp - H*th ; alternate engine for load balance
            tt = work.tile([R, CQ], F32, tag="tt")
            if (c * D + d) % 2 == 0:
                nc.scalar.activation(out=tt, in_=th, func=AF.Identity,
                                     scale=hn[:, d:d + 1], bias=cp[:, d:d + 1])
            else:
                nc.vector.tensor_scalar(tt, th, hn[:, d:d + 1], cp[:, d:d + 1],
                                        OP.mult, OP.add)
            # z = s2*T (into th's storage); PE accumulates z - T
            if (c * D + d) % 4 == 0:
                nc.gpsimd.tensor_tensor(th, eb, tt, OP.mult)  # th := z
            else:
                nc.vector.tensor_mul(th, eb, tt)  # th := z
            for pc in range(CQ // MMF):
                sl = slice(pc * MMF, (pc + 1) * MMF)
                nc.tensor.matmul(acc[:, sl], m_pos, th[:, sl],
                                 start=(d == 0), stop=False)
                nc.tensor.matmul(acc[:, sl], m_neg, tt[:, sl],
                                 start=False, stop=(d == D - 1))
        osb = outp.tile([R, CQ], F32, tag="osb")
        nc.scalar.activation(out=osb, in_=acc, func=AF.Identity,
                             scale=1.0 / D, bias=c2s[:, 0:1])
        nc.scalar.dma_start(out=out[c, :, :], in_=osb)


def _host_constants(betas, temperature, routes):
    """Build the 6 gather matrices (+-I) from the tiny replicated inputs."""
    betas = np.asarray(betas, dtype=np.float32)
    routes = np.asarray(routes).astype(np.int64)
    temp = np.abs(np.asarray(temperature, dtype=np.float32).reshape(-1)[0])
    scale = np.float32(1.0) / (np.sqrt(np.float32(EXPERT_DIM)) * temp)

    self_idx = np.arange(E)
    gate = np.where(
        routes == self_idx[:, None], np.float32(1.0),
        (np.float32(1.0) / (np.float32(1.0) +
                            np.exp(-betas[self_idx[:, None], routes]))),
    ).astype(np.float32)  # [E, W]

    A = np.zeros((W, R, R), dtype=np.float32)   # s_w gather (scale*beta folded)
    G = np.zeros((W, R, R), dtype=np.float32)   # v_w gather (1/P folded)
    rows = np.arange(R)
    e_of_r = rows // BC
    b_of_r = rows % BC
    for w in range(W):
        cols = routes[e_of_r, w] * BC + b_of_r
        A[w, rows, cols] += scale * gate[e_of_r, w]
        G[w, rows, cols] += np.float32(1.0 / P)

    m_d01h = 0.5 * (A[0] - A[1])
    m_d02 = A[0] - A[2]
    m_d12 = A[1] - A[2]
    m_h = 0.5 * (G[0] - G[1])
    m_hn = -m_h
    m_cp = (G[2] - G[1]) - m_h
    m_v2 = G[2]
    eye = np.eye(R, dtype=np.float32)
    mats = np.stack([m_d01h.T, m_d02.T, m_d12.T, m_hn.T, m_cp.T, m_v2.T,
                     eye, -eye]).astype(np.float32)
    return np.ascontiguousarray(mats)


_CACHE = {}


def kernel(Q, K, V, betas, temperature, routes, num_patches):
    Q = np.asarray(Q, dtype=np.float32)
    K = np.asarray(K, dtype=np.float32)
    V = np.asarray(V, dtype=np.float32)
    mats = _host_constants(betas, temperature, routes)

    if "nc" not in _CACHE:
        _CACHE["nc"] = _build_bass()
    nc = _CACHE["nc"]

    def shard(X, C):
        # [D,E,B,P] batch-slice -> chunk-major [D, P//C, R, C], contiguous
        outs = []
        for i in range(NCORES):
            sl = X[:, :, i * BC:(i + 1) * BC, :].reshape(D, R, P // C, C)
            outs.append(np.ascontiguousarray(sl.transpose(0, 2, 1, 3)))
        return outs

    qs, ks, vs = shard(Q, CQ), shard(K, CKV), shard(V, CKV)
    in_maps = [{"q": qs[i], "k": ks[i], "v": vs[i], "mats": mats}
               for i in range(NCORES)]

    res = bass_utils.run_bass_kernel_spmd(nc, in_maps,
                                          core_ids=list(range(NCORES)))
    _CACHE["last"] = res
    # device out: [P//CQ, R, CQ] with r = e*BC + b -> [BC, E*P]
    full = np.empty((B, E * P), dtype=np.float32)
    nq = P // CQ
    for i in range(NCORES):
        o = res.results[i]["out"].reshape(nq, E, BC, CQ)
        full[i * BC:(i + 1) * BC] = (
            o.transpose(2, 1, 0, 3).reshape(BC, E * P))
    return full


# revision 29
# speedup vs baseline: 1.1507x; 1.0867x over previous
"""Trainium2 Bass kernel for CantorGlobalAttention.

Math (per direction d, expert e, batch b, patch p):
  s_w   = Ksum[d, routes[e,w], b] * scale * beta[e,w]      (w = 0..2)
  v_w   = Vmean[d, routes[e,w], b]
  out_d = softmax_w(q * s_w) . v
Final output = mean over d, laid out [B, E*P].

Since the softmax is over only W=3 scalars that multiply the same q, it
collapses to an elementwise function of q with 6 per-row constants:
  sg  = sigmoid(q*(s0-s1)) = 0.5 + 0.5*tanh(q*(s0-s1)/2)
  A   = v1 + (v0-v1)*sg                       (exact 2-way softmax of {0,1})
  s2  = 1/(1 + e^{q*(s0-s2)} + e^{q*(s1-s2)}) (true 3-way weight of w=2)
  out_d = A + (v2 - A)*s2
Rewritten with tanh (so all ACT funcs live in one table set):
  H  = 0.5*(v0-v1);  Cp = (v2-v1) - H;  T = Cp - H*tanh(q*d01h)
  out_d = v2 - T + T*s2
Accumulated over d on the PE via identity matmuls into PSUM:
  OUT = 0.2 * (sum_d (z_d - T_d)) + 0.2*sum_d v2_d,   z = T*s2

All the per-row constants are produced on-device by tiny one-hot matmuls
(host-built gather matrices folded with scale/beta/0.5 factors) applied to
Ksum / Vsum columns (one col per direction).

Sharding: data-parallel over batch (dim 2), 8 cores x 8 batches.
"""

import numpy as np
from contextlib import ExitStack

import concourse.bass as bass
import concourse.bacc as bacc
import concourse.tile as tile
from concourse import mybir
from concourse import bass_utils

F32 = mybir.dt.float32
AF = mybir.ActivationFunctionType
OP = mybir.AluOpType

D, E, B, P = 5, 16, 64, 4096
W = 3
NCORES = 8
BC = B // NCORES          # 8 batches per core
R = E * BC                # 128 rows = partitions, r = e*BC + b
EXPERT_DIM = 128

CKV = 1024                # K/V reduce chunk (cols)
CQ = 2048                 # Q-phase chunk (cols)
MMF = 512                 # matmul max free dim

CLAMP = 1e37              # keep reciprocal_approx_fast input finite


def _build_bass():
    nc = bacc.Bacc("TRN2", debug=False, num_devices=NCORES)
    # chunk-major layouts: every tile transfer is one contiguous DRAM block
    q = nc.dram_tensor("q", [D, P // CQ, R, CQ], F32,
                       kind="ExternalInput").ap()
    k = nc.dram_tensor("k", [D, P // CKV, R, CKV], F32,
                       kind="ExternalInput").ap()
    v = nc.dram_tensor("v", [D, P // CKV, R, CKV], F32,
                       kind="ExternalInput").ap()
    # 6 gather matrices (pre-transposed for lhsT) + I and -I
    mats = nc.dram_tensor("mats", [8, R, R], F32, kind="ExternalInput").ap()
    out = nc.dram_tensor("out", [P // CQ, R, CQ], F32,
                         kind="ExternalOutput").ap()

    with ExitStack() as ctx:
        tc = ctx.enter_context(tile.TileContext(nc))
        _body(ctx, tc, q, k, v, mats, out)
    if not nc.is_finalized():
        nc.finalize()
    return nc


def _body(ctx, tc, q, k, v, mats, out):
    nc = tc.nc
    singles = ctx.enter_context(tc.tile_pool(name="singles", bufs=1))

    # Long-lived Q-phase pools are created BEFORE the short-lived K/V pool:
    # the stack allocator then never hands the K/V zone to Q-phase tiles,
    # which would serialize the Q pipeline behind the last V reduce
    # (released-zone dependency).
    qpool = ctx.enter_context(tc.tile_pool(name="qp", bufs=5))
    work = ctx.enter_context(tc.tile_pool(name="work", bufs=3))
    th_pool = ctx.enter_context(tc.tile_pool(name="thp", bufs=3))
    outp = ctx.enter_context(tc.tile_pool(name="outp", bufs=2))

    # All input loads go through the SP trigger stream: its FIFO gives
    # strict priority ordering (K+Qpre interleaved per d -> V -> Q rest) and
    # the DGE spreads transfers over all 16 HW queues regardless of engine.
    def load(dst, src_ap):
        return nc.sync.dma_start(out=dst, in_=src_ap)

    # --- constants to SBUF ---
    mat_sb = []
    for i in range(8):
        m = singles.tile([R, R], F32, tag=f"mat{i}")
        load(m, mats[i, :, :])
        mat_sb.append(m)
    (m_d01h, m_d02, m_d12, m_hn, m_cp, m_v2, m_pos, m_neg) = mat_sb

    nkv = P // CKV
    # per-direction constants ([R,1] tiles, one set per d)
    d01h = [singles.tile([R, 1], F32, tag=f"d01h{d}", name=f"d01h{d}")
            for d in range(D)]
    d02 = [singles.tile([R, 1], F32, tag=f"d02{d}", name=f"d02{d}")
            for d in range(D)]
    d12 = [singles.tile([R, 1], F32, tag=f"d12{d}", name=f"d12{d}")
            for d in range(D)]
    hn = [singles.tile([R, 1], F32, tag=f"hn{d}", name=f"hn{d}")
            for d in range(D)]
    cp = [singles.tile([R, 1], F32, tag=f"cp{d}", name=f"cp{d}")
            for d in range(D)]
    v2 = [singles.tile([R, 1], F32, tag=f"v2{d}", name=f"v2{d}")
            for d in range(D)]
    c2s = singles.tile([R, 1], F32, tag="c2s")
    qpre = []

    with tc.tile_pool(name="kv", bufs=4) as kv_pool, \
         tc.tile_pool(name="part", bufs=4) as part_pool, \
         tc.tile_pool(name="prep", bufs=4, space="PSUM") as pre_psum:

        def reduce_d(name, src, d, sum_col, act_share):
            # reduce src[d] (nkv contiguous chunks) into sum_col [R,1]
            parts = part_pool.tile([R, nkv], F32, tag=f"{name}p")
            for c in range(nkv):
                t = kv_pool.tile([R, CKV], F32, tag=name)
                load(t, src[d, c, :, :])
                if act_share and c % 2 == 0:
                    nc.scalar.activation(out=t, in_=t, func=AF.Copy,
                                         accum_out=parts[:, c:c + 1])
                else:
                    nc.vector.tensor_reduce(out=parts[:, c:c + 1], in_=t,
                                            axis=mybir.AxisListType.X,
                                            op=OP.add)
            nc.vector.tensor_add(parts[:, 0:1], parts[:, 0:1], parts[:, 1:2])
            nc.vector.tensor_add(parts[:, 2:3], parts[:, 2:3], parts[:, 3:4])
            nc.vector.tensor_add(sum_col, parts[:, 0:1], parts[:, 2:3])

        def prelude_d(sum_col, pairs):
            for lhsT, dst in pairs:
                pt = pre_psum.tile([R, 1], F32, tag="pre")
                nc.tensor.matmul(pt, lhsT, sum_col, start=True, stop=True)
                nc.vector.tensor_copy(dst, pt)

        # K + Q-prefetch interleaved per direction: compute can start as
        # soon as direction 0 has landed
        for d in range(D):
            ks = singles.tile([R, 1], F32, tag=f"ks{d}")
            reduce_d("k", k, d, ks, act_share=False)
            qt = qpool.tile([R, CQ], F32, tag="q")
            load(qt, q[d, 0, :, :])
            qpre.append(qt)
            prelude_d(ks, ((m_d01h, d01h[d]), (m_d02, d02[d]),
                           (m_d12, d12[d])))

        # V per direction (reduces split ACT/DVE)
        for d in range(D):
            vs = singles.tile([R, 1], F32, tag=f"vs{d}")
            reduce_d("v", v, d, vs, act_share=True)
            prelude_d(vs, ((m_hn, hn[d]), (m_cp, cp[d]), (m_v2, v2[d])))

    nc.vector.tensor_add(c2s, v2[0], v2[1])
    nc.vector.tensor_add(c2s, c2s, v2[2])
    nc.vector.tensor_add(c2s, c2s, v2[3])
    nc.vector.tensor_add(c2s, c2s, v2[4])
    nc.vector.tensor_scalar_mul(c2s, c2s, 1.0 / D)

    # --- Q phase ---
    acc_pool = ctx.enter_context(tc.tile_pool(name="accp", bufs=2,
                                              space="PSUM"))
    nq = P // CQ
    for c in range(nq):
        acc = acc_pool.tile([R, CQ], F32, tag="acc")
        for d in range(D):
            if c == 0:
                qt = qpre[d]
            else:
                qt = qpool.tile([R, CQ], F32, tag="q")
                nc.sync.dma_start(out=qt, in_=q[d, c, :, :])
            th = th_pool.tile([R, CQ], F32, tag="th")
            nc.scalar.activation(out=th, in_=qt, func=AF.Tanh, scale=d01h[d])
            ea = work.tile([R, CQ], F32, tag="ea")
            nc.scalar.activation(out=ea, in_=qt, func=AF.Exp, scale=d02[d])
            eb = work.tile([R, CQ], F32, tag="eb")
            nc.scalar.activation(out=eb, in_=qt, func=AF.Exp, scale=d12[d])
            # s2 = 1/min(1 + ea + eb, CLAMP); reuse ea/eb storage in place
            nc.gpsimd.tensor_tensor(ea, ea, eb, OP.add)
            nc.vector.tensor_scalar(ea, ea, 1.0, CLAMP, OP.add, OP.min)
            nc.vector.reciprocal_approx_fast(out=eb, in_=ea)  # eb := s2
            # T = Cp - H*th ; alternate engine for load balance
            tt = work.tile([R, CQ], F32, tag="tt")
            if (c * D + d) % 2 == 0:
                nc.scalar.activation(out=tt, in_=th, func=AF.Identity,
                                     scale=hn[d], bias=cp[d])
            else:
                nc.vector.tensor_scalar(tt, th, hn[d], cp[d],
                                        OP.mult, OP.add)
            # z = s2*T (into th's storage); PE accumulates z - T
            if (c * D + d) % 4 == 0:
                nc.gpsimd.tensor_tensor(th, eb, tt, OP.mult)  # th := z
            else:
                nc.vector.tensor_mul(th, eb, tt)  # th := z
            for pc in range(CQ // MMF):
                sl = slice(pc * MMF, (pc + 1) * MMF)
                nc.tensor.matmul(acc[:, sl], m_pos, th[:, sl],
                                 start=(d == 0), stop=False)
                nc.tensor.matmul(acc[:, sl], m_neg, tt[:, sl],
                                 start=False, stop=(d == D - 1))
        osb = outp.tile([R, CQ], F32, tag="osb")
        nc.scalar.activation(out=osb, in_=acc, func=AF.Identity,
                             scale=1.0 / D, bias=c2s[:, 0:1])
        nc.scalar.dma_start(out=out[c, :, :], in_=osb)


def _host_constants(betas, temperature, routes):
    """Build the 6 gather matrices (+-I) from the tiny replicated inputs."""
    betas = np.asarray(betas, dtype=np.float32)
    routes = np.asarray(routes).astype(np.int64)
    temp = np.abs(np.asarray(temperature, dtype=np.float32).reshape(-1)[0])
    scale = np.float32(1.0) / (np.sqrt(np.float32(EXPERT_DIM)) * temp)

    self_idx = np.arange(E)
    gate = np.where(
        routes == self_idx[:, None], np.float32(1.0),
        (np.float32(1.0) / (np.float32(1.0) +
                            np.exp(-betas[self_idx[:, None], routes]))),
    ).astype(np.float32)  # [E, W]

    A = np.zeros((W, R, R), dtype=np.float32)   # s_w gather (scale*beta folded)
    G = np.zeros((W, R, R), dtype=np.float32)   # v_w gather (1/P folded)
    rows = np.arange(R)
    e_of_r = rows // BC
    b_of_r = rows % BC
    for w in range(W):
        cols = routes[e_of_r, w] * BC + b_of_r
        A[w, rows, cols] += scale * gate[e_of_r, w]
        G[w, rows, cols] += np.float32(1.0 / P)

    m_d01h = 0.5 * (A[0] - A[1])
    m_d02 = A[0] - A[2]
    m_d12 = A[1] - A[2]
    m_h = 0.5 * (G[0] - G[1])
    m_hn = -m_h
    m_cp = (G[2] - G[1]) - m_h
    m_v2 = G[2]
    eye = np.eye(R, dtype=np.float32)
    mats = np.stack([m_d01h.T, m_d02.T, m_d12.T, m_hn.T, m_cp.T, m_v2.T,
                     eye, -eye]).astype(np.float32)
    return np.ascontiguousarray(mats)


_CACHE = {}


def kernel(Q, K, V, betas, temperature, routes, num_patches):
    Q = np.asarray(Q, dtype=np.float32)
    K = np.asarray(K, dtype=np.float32)
    V = np.asarray(V, dtype=np.float32)
    mats = _host_constants(betas, temperature, routes)

    if "nc" not in _CACHE:
        _CACHE["nc"] = _build_bass()
    nc = _CACHE["nc"]

    def shard(X, C):
        # [D,E,B,P] batch-slice -> chunk-major [D, P//C, R, C], contiguous
        outs = []
        for i in range(NCORES):
            sl = X[:, :, i * BC:(i + 1) * BC, :].reshape(D, R, P // C, C)
            outs.append(np.ascontiguousarray(sl.transpose(0, 2, 1, 3)))
        return outs

    qs, ks, vs = shard(Q, CQ), shard(K, CKV), shard(V, CKV)
    in_maps = [{"q": qs[i], "k": ks[i], "v": vs[i], "mats": mats}
               for i in range(NCORES)]

    res = bass_utils.run_bass_kernel_spmd(nc, in_maps,
                                          core_ids=list(range(NCORES)))
    _CACHE["last"] = res
    # device out: [P//CQ, R, CQ] with r = e*BC + b -> [BC, E*P]
    full = np.empty((B, E * P), dtype=np.float32)
    nq = P // CQ
    for i in range(NCORES):
        o = res.results[i]["out"].reshape(nq, E, BC, CQ)
        full[i * BC:(i + 1) * BC] = (
            o.transpose(2, 1, 0, 3).reshape(BC, E * P))
    return full
